# revision 1
# baseline (speedup 1.0000x reference)
"""Trainium2 Bass kernel for nn_MultiHeadAttention (B=1, S=4096, D=2048, H=16, HD=128).

Sharding: tensor-parallel over heads — 2 heads per core on 8 NeuronCores.
Each core computes its 2 heads' Q/K/V projections, causal attention, and a
partial output projection (row-split Wo); the host sums the 8 partials and
adds the output bias (the all-reduce/unshard step).

Layout strategy (per core, all matmuls bf16 with fp32 PSUM accumulation):
  - X^T [2048, 4096] uploaded (e-major) so projections contract over e.
  - Q, K are produced transposed: QT/KT [d, s]. Scores are computed
    transposed, S^T[k, q] = KT_tile^T @ QT, so that:
      * p = exp(S^T) tiles have k on partitions -> attn@V needs no transpose:
        outT[d, q] accumulates lhsT=V_tile[k, d], rhs=p[k, q]
      * softmax denominators come from a ones-column matmul accumulated in
        PSUM alongside the attn@V accumulation
  - V is produced natural [s, d] (X^T tiles as stationary operand).
  - 1/denom is broadcast across partitions with a tiny ones-row matmul and
    fused into the PSUM->SBUF copy of outT.
  - O-projection: out[s, e] += outT_h[d, s]^T @ WoT_h[d, e], accumulated over
    both local heads; result DMA'd out as the fp32 partial.

Build notes:
  - Built with bacc.Bacc: walrus encodes at most ONE sem wait per
    instruction, and Bacc's generate_event_semaphores pass splits larger
    wait sets into event-semaphore chains.
  - Attention runs qb-outer with both heads' k-groups interleaved and the
    O-projection fused per q-block, so the scheduler always has independent
    matmuls to fill pipeline bubbles.
  - Denominator matmuls are packed 4-per-PSUM-bank via tile_position column
    groups; the 4 partial rows are combined on DVE before the reciprocal.
Measured on TRN2: ~625-635 us/core HW exec, rel err ~6e-3 vs fp32 reference.
"""

import numpy as np
import ml_dtypes

import concourse.bass as bass
import concourse.mybir as mybir
import concourse.tile as tile
from concourse import bacc
from concourse.bass_utils import run_bass_kernel_spmd


S = 4096          # sequence length
D = 2048          # model dim
NCORES = 8
DL = D // NCORES  # 256 local head dims (2 heads)
NH = 2            # heads per core
HD = 128          # head dim
QB = 512          # q block width
NQB = S // QB     # 8
KT = 128          # k tile (partitions)
NKT = S // KT     # 32
ET = 128          # e contraction tile
NET = D // ET     # 16
NST = S // 128    # 32 s-tiles
SCALE = 1.0 / np.sqrt(HD)

BF16 = mybir.dt.bfloat16
F32 = mybir.dt.float32


def build_nc(is_causal: bool) -> bass.Bass:
    # Bacc (not raw Bass): its finalize() pipeline splits multi-sem sync
    # waits into event-semaphore chains — walrus encodes at most one wait
    # per instruction.
    nc = bacc.Bacc()

    XT = nc.dram_tensor("xt", [D, S], BF16, kind="ExternalInput")
    WQT = nc.dram_tensor("wqt", [D, DL], BF16, kind="ExternalInput")
    WKT = nc.dram_tensor("wkt", [D, DL], BF16, kind="ExternalInput")
    WVT = nc.dram_tensor("wvt", [D, DL], BF16, kind="ExternalInput")
    # bias columns [128, 4]: bq.d0 | bq.d1 | bk.d0 | bk.d1
    BQKC = nc.dram_tensor("bqkc", [128, 4], F32, kind="ExternalInput")
    BVROW = nc.dram_tensor("bvrow", [1, DL], BF16, kind="ExternalInput")
    WOT = nc.dram_tensor("wot", [DL, D], BF16, kind="ExternalInput")
    # masks[0..3]: additive causal masks (0 / -1e9); masks[4][:, :128]: identity
    MASKS = nc.dram_tensor("masks", [5, 128, QB], BF16, kind="ExternalInput")
    OUT = nc.dram_tensor("out", [S, D], F32, kind="ExternalOutput")

    with tile.TileContext(nc) as tc:
        with tc.tile_pool(name="persist", bufs=1) as persist:
            # Q head0 | Q head1 | K head0 | K head1, each [128, 4096]
            qkt = persist.tile([128, 4 * S], BF16, name="qkt")
            # V natural layout: s-tile st at cols [st*256, (st+1)*256), head h at +h*128
            vt = persist.tile([128, NST * DL], BF16, name="vt")
            ones_col = persist.tile([128, 1], BF16, name="ones_col")
            ones_row = persist.tile([1, 512], BF16, name="ones_row")
            biasqk = persist.tile([128, 4], F32, name="biasqk")
            bvrow_sb = persist.tile([1, DL], BF16, name="bvrow_sb")
            bvb_sb = persist.tile([128, DL], BF16, name="bvb_sb")

            nc.vector.memset(ones_col[:, :], 1.0)
            nc.vector.memset(ones_row[:, :], 1.0)
            nc.sync.dma_start(out=bvrow_sb[:, :], in_=BVROW[:, :])
            nc.sync.dma_start(out=biasqk[:, :], in_=BQKC[:, :])

            # ---------------- Phase 2: QKV projections ----------------
            with tc.tile_pool(name="xtp", bufs=1) as xtp, \
                 tc.tile_pool(name="wp", bufs=1) as wp, \
                 tc.tile_pool(name="ps2", bufs=2, space="PSUM") as ps2:
                xt_sb = xtp.tile([128, NET * S], BF16, name="xt_sb")
                # V weights first so the first V matmuls aren't queued behind
                # the 16 MB X^T stream
                wv_sb = wp.tile([128, NET * DL], BF16, name="wv_sb", tag="wv")
                for et in range(NET):
                    nc.sync.dma_start(
                        out=wv_sb[:, et * DL : (et + 1) * DL],
                        in_=WVT[et * 128 : (et + 1) * 128, :],
                    )
                for et in range(NET):
                    nc.sync.dma_start(
                        out=xt_sb[:, et * S : (et + 1) * S],
                        in_=XT[et * 128 : (et + 1) * 128, :],
                    )
                # broadcast bv across partitions once; folded into each
                # V tile's PSUM->SBUF copy below
                psb = ps2.tile([128, DL], F32, name="psb", tag="psv")
                nc.tensor.matmul(
                    psb[:, :], lhsT=ones_row[:, :128], rhs=bvrow_sb[:, :],
                    start=True, stop=True,
                )
                nc.vector.tensor_copy(bvb_sb[:, :], psb[:, :])
                for st in range(NST):
                    psv = ps2.tile([128, DL], F32, name="psv", tag="psv")
                    for et in range(NET):
                        nc.tensor.matmul(
                            psv[:, :],
                            lhsT=xt_sb[:, et * S + st * 128 : et * S + (st + 1) * 128],
                            rhs=wv_sb[:, et * DL : (et + 1) * DL],
                            start=(et == 0),
                            stop=(et == NET - 1),
                        )
                    nc.vector.scalar_tensor_tensor(
                        out=vt[:, st * DL : (st + 1) * DL],
                        in0=psv[:, :],
                        scalar=1.0,
                        in1=bvb_sb[:, :],
                        op0=mybir.AluOpType.mult,
                        op1=mybir.AluOpType.add,
                    )

                # K then Q (transposed [d, s]): lhsT = W^T slice, rhs = X^T
                for t_idx, (wdram, base4, bias_base) in enumerate(
                    [(WKT, 2, 2), (WQT, 0, 0)]
                ):
                    w_sb = wp.tile([128, NET * DL], BF16, name="w_sb", tag=f"w{t_idx}")
                    for et in range(NET):
                        nc.sync.dma_start(
                            out=w_sb[:, et * DL : (et + 1) * DL],
                            in_=wdram[et * 128 : (et + 1) * 128, :],
                        )
                    for dt in range(NH):
                        for sb in range(NQB):
                            psq = ps2.tile([128, QB], F32, name="psq", tag="psq")
                            for et in range(NET):
                                nc.tensor.matmul(
                                    psq[:, :],
                                    lhsT=w_sb[:, et * DL + dt * 128 : et * DL + (dt + 1) * 128],
                                    rhs=xt_sb[:, et * S + sb * QB : et * S + (sb + 1) * QB],
                                    start=(et == 0),
                                    stop=(et == NET - 1),
                                )
                            nc.vector.tensor_scalar_add(
                                out=qkt[:, (base4 + dt) * S + sb * QB : (base4 + dt) * S + (sb + 1) * QB],
                                in0=psq[:, :],
                                scalar1=biasqk[:, bias_base + dt : bias_base + dt + 1],
                            )

            # ---------------- Phases 3+4: attention, then O-projection ----
            with tc.tile_pool(name="mid", bufs=1) as mid, \
                 tc.tile_pool(name="psO", bufs=2, space="PSUM") as psO_p, \
                 tc.tile_pool(name="psD", bufs=2, space="PSUM") as psD_p, \
                 tc.tile_pool(name="psS", bufs=3, space="PSUM") as psS_p, \
                 tc.tile_pool(name="psF", bufs=1, space="PSUM") as psF_p, \
                 tc.tile_pool(name="pp", bufs=10) as pp, \
                 tc.tile_pool(name="rp", bufs=3) as rp, \
                 tc.tile_pool(name="op", bufs=4) as op:
                # normalized attention outputs, transposed: (h*NQB+qb) tile [128d, 512q]
                outt = mid.tile([128, NH * NQB * QB], BF16, name="outt")
                wot_sb = mid.tile([128, NH * D], BF16, name="wot_sb")
                masks_sb = mid.tile([128, 5 * QB], BF16, name="masks_sb")
                for h in range(NH):
                    nc.sync.dma_start(
                        out=wot_sb[:, h * D : (h + 1) * D],
                        in_=WOT[h * 128 : (h + 1) * 128, :],
                    )
                if is_causal:
                    for jj in range(5):
                        nc.sync.dma_start(
                            out=masks_sb[:, jj * QB : (jj + 1) * QB],
                            in_=MASKS[jj, :, :],
                        )

                # qb-outer; the two heads' k-groups are interleaved so the
                # scheduler always has a second independent pipeline to fill
                # PE/ACT bubbles. O-projection fused per q-block.
                for qb in range(NQB):
                    kmax = 4 * (qb + 1) if is_causal else NKT
                    groups = list(range(0, kmax, 4))
                    ngroups = len(groups)
                    psO = {}
                    psD = {}
                    for h in range(NH):
                        psO[h] = psO_p.tile([128, QB], F32, name="psO", tag="psO")
                        psD[h] = psD_p.tile([128, QB], F32, name="psD", tag="psD")
                    for gi, kg in enumerate(groups):
                        for h in range(NH):
                            ps_group = []
                            for jj4 in range(4):
                                kt = kg + jj4
                                psS = psS_p.tile([128, QB], F32, name="psS", tag="psS")
                                diag = is_causal and kt >= 4 * qb
                                nc.tensor.matmul(
                                    psS[:, :],
                                    lhsT=qkt[:, (2 + h) * S + kt * 128 : (2 + h) * S + (kt + 1) * 128],
                                    rhs=qkt[:, h * S + qb * QB : h * S + (qb + 1) * QB],
                                    start=True,
                                    stop=not diag,
                                )
                                if diag:
                                    # += additive causal mask (0 / -1e9) via an
                                    # identity-weight matmul: stays on PE, no
                                    # cross-engine hop before exp
                                    jj = kt - 4 * qb
                                    nc.tensor.matmul(
                                        psS[:, :],
                                        lhsT=masks_sb[:, 4 * QB : 4 * QB + 128],
                                        rhs=masks_sb[:, jj * QB : (jj + 1) * QB],
                                        start=False,
                                        stop=True,
                                    )
                                p = pp.tile([128, QB], BF16, name="p", tag="p")
                                nc.scalar.activation(
                                    p[:, :], psS[:, :],
                                    mybir.ActivationFunctionType.Exp,
                                    scale=float(SCALE),
                                )
                                ps_group.append(p)
                            for jj4 in range(4):
                                kt = kg + jj4
                                nc.tensor.matmul(
                                    psO[h][:, :],
                                    lhsT=vt[:, kt * DL + h * 128 : kt * DL + (h + 1) * 128],
                                    rhs=ps_group[jj4][:, :],
                                    start=(gi == 0 and jj4 == 0),
                                    stop=(gi == ngroups - 1 and jj4 == 3),
                                )
                            # 4 packed denominator matmuls into separate PE
                            # column groups / PSUM rows
                            for jj4 in range(4):
                                nc.tensor.matmul(
                                    psD[h][32 * jj4 : 32 * jj4 + 1, :],
                                    lhsT=ones_col[:, :],
                                    rhs=ps_group[jj4][:, :],
                                    start=(gi == 0),
                                    stop=(gi == ngroups - 1),
                                    tile_position=(0, 32 * jj4),
                                )
                    for h in range(NH):
                        # combine the 4 partial-sum rows, then 1/denom
                        sum4 = rp.tile([1, QB], F32, name="sum4", tag="sum4")
                        nc.vector.tensor_copy(sum4[:, :], psD[h][0:1, :])
                        for jj4 in range(1, 4):
                            nc.vector.scalar_tensor_tensor(
                                out=sum4[:, :],
                                in0=psD[h][32 * jj4 : 32 * jj4 + 1, :],
                                scalar=1.0,
                                in1=sum4[:, :],
                                op0=mybir.AluOpType.mult,
                                op1=mybir.AluOpType.add,
                            )
                        recip = rp.tile([1, QB], BF16, name="recip", tag="recip")
                        with nc.allow_low_precision(reason="softmax recip in bf16"):
                            nc.vector.reciprocal(recip[:, :], sum4[:, :])
                        # broadcast 1/denom across partitions, reusing psD
                        nc.tensor.matmul(
                            psD[h][:, :], lhsT=ones_row[:, :128], rhs=recip[:, :],
                            start=True, stop=True,
                        )
                        rb = rp.tile([128, QB], F32, name="rb", tag="rb")
                        nc.vector.tensor_copy(rb[:, :], psD[h][:, :])
                        o_base = (h * NQB + qb) * QB
                        nc.vector.tensor_mul(
                            outt[:, o_base : o_base + QB], psO[h][:, :], rb[:, :]
                        )

                    # O-projection for this q-block (both heads ready)
                    for j in range(4):
                        st = qb * 4 + j
                        for et in range(4):
                            psF = psF_p.tile([128, 512], F32, name="psF", tag="psF")
                            for h in range(NH):
                                o_base = (h * NQB + qb) * QB + j * 128
                                nc.tensor.matmul(
                                    psF[:, :],
                                    lhsT=outt[:, o_base : o_base + 128],
                                    rhs=wot_sb[:, h * D + et * 512 : h * D + (et + 1) * 512],
                                    start=(h == 0),
                                    stop=(h == NH - 1),
                                )
                            osb = op.tile([128, 512], F32, name="osb", tag="osb")
                            nc.vector.tensor_copy(osb[:, :], psF[:, :])
                            nc.sync.dma_start(
                                out=OUT[st * 128 : (st + 1) * 128, et * 512 : (et + 1) * 512],
                                in_=osb[:, :],
                            )
    nc.finalize()
    return nc


def _bf16(a: np.ndarray) -> np.ndarray:
    return np.ascontiguousarray(a.astype(ml_dtypes.bfloat16))


def make_in_maps(X, Wq, bq, Wk, bk, Wv, bv, Wo, is_causal: bool):
    x2d = np.asarray(X, dtype=np.float32).reshape(S, D)
    xt = _bf16(x2d.T)
    masks = np.zeros((5, 128, QB), dtype=ml_dtypes.bfloat16)
    if is_causal:
        ki = np.arange(128)[:, None]
        qj = np.arange(QB)[None, :]
        for jj in range(4):
            masks[jj] = np.where(128 * jj + ki <= qj, 0.0, -1e9).astype(
                ml_dtypes.bfloat16
            )
        masks[4][:, :128] = np.eye(128, dtype=ml_dtypes.bfloat16)

    in_maps = []
    for c in range(NCORES):
        sl = slice(c * DL, (c + 1) * DL)
        in_maps.append(
            {
                "xt": xt,
                "wqt": _bf16(np.asarray(Wq)[sl, :].T),
                "wkt": _bf16(np.asarray(Wk)[sl, :].T),
                "wvt": _bf16(np.asarray(Wv)[sl, :].T),
                "bqkc": np.ascontiguousarray(
                    np.stack(
                        [
                            np.asarray(bq, dtype=np.float32)[sl][:128],
                            np.asarray(bq, dtype=np.float32)[sl][128:],
                            np.asarray(bk, dtype=np.float32)[sl][:128],
                            np.asarray(bk, dtype=np.float32)[sl][128:],
                        ],
                        axis=1,
                    )
                ),
                "bvrow": _bf16(np.asarray(bv)[None, sl]),
                "wot": _bf16(np.asarray(Wo)[:, sl].T),
                "masks": masks,
            }
        )
    return in_maps


_NC_CACHE: dict = {}


def _get_nc(is_causal: bool) -> bass.Bass:
    if is_causal not in _NC_CACHE:
        _NC_CACHE[is_causal] = build_nc(is_causal)
    return _NC_CACHE[is_causal]


def kernel(X, Wq, bq, Wk, bk, Wv, bv, Wo, bo, is_causal, **run_kwargs):
    causal = bool(int(np.asarray(is_causal)))
    nc = _get_nc(causal)
    in_maps = make_in_maps(X, Wq, bq, Wk, bk, Wv, bv, Wo, causal)
    res = run_bass_kernel_spmd(nc, in_maps, core_ids=list(range(NCORES)), **run_kwargs)
    out = np.asarray(bo, dtype=np.float32)[None, :].repeat(S, axis=0)
    for c in range(NCORES):
        out += res.results[c]["out"]
    return out.reshape(1, S, D)



# revision 2
# speedup vs baseline: 1.1288x; 1.1288x over previous
"""Trainium2 Bass kernel for nn_MultiHeadAttention (B=1, S=4096, D=2048, H=16, HD=128).

Sharding: tensor-parallel over heads — 2 heads per core on 8 NeuronCores.
Each core computes its 2 heads' Q/K/V projections, causal attention, and a
partial output projection (row-split Wo); the host sums the 8 partials and
adds the output bias (the all-reduce/unshard step).

Layout strategy (per core, all matmuls bf16 with fp32 PSUM accumulation):
  - X^T [2048, 4096] streamed in four 1024-column quarters (double-buffered)
    with all projection weights preloaded, so the first matmuls start after
    ~5 MB of DMA instead of the full 16 MB X^T stream.
  - Q, K are produced transposed: QT/KT [d, s]. Scores are computed
    transposed, S^T[k, q] = KT_tile^T @ QT, so that p = exp(S^T) tiles have
    k on partitions -> attn@V needs no transpose.
  - Causal masking is multiplicative (0/1) on DVE after the exp, removing
    the identity-weight mask matmuls from the PE stream.
  - Softmax denominators: ones-column matmuls packed 4-per-PSUM-bank via
    tile_position; the 4 partial rows are combined on DVE, inverted with
    reciprocal_approx_fast, and broadcast across partitions on GpSimd
    (partition_broadcast) — no PE broadcast matmul, no slow DVE reciprocal.
  - O-projection: out[s, e] += outT_h[d, s]^T @ WoT_h[d, e], accumulated over
    both local heads; result DMA'd out as the fp32 partial.

Build notes:
  - Built with bacc.Bacc: walrus encodes at most ONE sem wait per
    instruction, and Bacc's generate_event_semaphores pass splits larger
    wait sets into event-semaphore chains.
  - Attention runs qb-outer with both heads' k-groups interleaved and the
    O-projection fused per q-block, so the scheduler always has independent
    matmuls to fill pipeline bubbles.
"""

import numpy as np
import ml_dtypes

import concourse.bass as bass
import concourse.mybir as mybir
import concourse.tile as tile
from concourse import bacc
from concourse.bass_utils import run_bass_kernel_spmd


S = 4096          # sequence length
D = 2048          # model dim
NCORES = 8
DL = D // NCORES  # 256 local head dims (2 heads)
NH = 2            # heads per core
HD = 128          # head dim
QB = 512          # q block width
NQB = S // QB     # 8
KT = 128          # k tile (partitions)
NKT = S // KT     # 32
ET = 128          # e contraction tile
NET = D // ET     # 16
NST = S // 128    # 32 s-tiles
SQ = 1024         # X^T streaming quarter width (s columns)
NSQ = S // SQ     # 4 quarters
SCALE = 1.0 / np.sqrt(HD)

BF16 = mybir.dt.bfloat16
F32 = mybir.dt.float32


def build_nc(is_causal: bool) -> bass.Bass:
    # Bacc (not raw Bass): its finalize() pipeline splits multi-sem sync
    # waits into event-semaphore chains — walrus encodes at most one wait
    # per instruction.
    nc = bacc.Bacc()

    XT = nc.dram_tensor("xt", [D, S], BF16, kind="ExternalInput")
    WQT = nc.dram_tensor("wqt", [D, DL], BF16, kind="ExternalInput")
    WKT = nc.dram_tensor("wkt", [D, DL], BF16, kind="ExternalInput")
    WVT = nc.dram_tensor("wvt", [D, DL], BF16, kind="ExternalInput")
    # bias columns [128, 4]: bq.d0 | bq.d1 | bk.d0 | bk.d1
    BQKC = nc.dram_tensor("bqkc", [128, 4], F32, kind="ExternalInput")
    BVROW = nc.dram_tensor("bvrow", [1, DL], BF16, kind="ExternalInput")
    WOT = nc.dram_tensor("wot", [DL, D], BF16, kind="ExternalInput")
    # masks[0..3]: multiplicative causal masks (1 below/on diagonal, 0 above)
    MASKS = nc.dram_tensor("masks", [4, 128, QB], BF16, kind="ExternalInput")
    OUT = nc.dram_tensor("out", [S, D], F32, kind="ExternalOutput")

    with tile.TileContext(nc) as tc:
        with tc.tile_pool(name="persist", bufs=1) as persist:
            # Q head0 | Q head1 | K head0 | K head1, each [128, 4096]
            qkt = persist.tile([128, 4 * S], BF16, name="qkt")
            # V natural layout: s-tile st at cols [st*256, (st+1)*256), head h at +h*128
            vt = persist.tile([128, NST * DL], BF16, name="vt")
            wot_sb = persist.tile([128, NH * D], BF16, name="wot_sb")
            masks_sb = persist.tile([128, 4 * QB], BF16, name="masks_sb")
            wv_sb = persist.tile([128, NET * DL], BF16, name="wv_sb")
            wk_sb = persist.tile([128, NET * DL], BF16, name="wk_sb")
            wq_sb = persist.tile([128, NET * DL], BF16, name="wq_sb")
            ones_col = persist.tile([128, 1], BF16, name="ones_col")
            biasqk = persist.tile([128, 4], F32, name="biasqk")
            bvrow_sb = persist.tile([1, DL], BF16, name="bvrow_sb")
            bvb_sb = persist.tile([128, DL], BF16, name="bvb_sb")

            nc.vector.memset(ones_col[:, :], 1.0)
            # weights first: V weights gate the first matmuls
            for w_sb, wdram in ((wv_sb, WVT), (wk_sb, WKT), (wq_sb, WQT)):
                for et in range(NET):
                    nc.sync.dma_start(
                        out=w_sb[:, et * DL : (et + 1) * DL],
                        in_=wdram[et * 128 : (et + 1) * 128, :],
                    )
            nc.sync.dma_start(out=bvrow_sb[:, :], in_=BVROW[:, :])
            nc.sync.dma_start(out=biasqk[:, :], in_=BQKC[:, :])
            for h in range(NH):
                nc.sync.dma_start(
                    out=wot_sb[:, h * D : (h + 1) * D],
                    in_=WOT[h * 128 : (h + 1) * 128, :],
                )
            if is_causal:
                for jj in range(4):
                    nc.sync.dma_start(
                        out=masks_sb[:, jj * QB : (jj + 1) * QB],
                        in_=MASKS[jj, :, :],
                    )
            # broadcast bv across partitions once; folded into each V tile's
            # PSUM->SBUF copy below
            nc.gpsimd.partition_broadcast(bvb_sb[:, :], bvrow_sb[:, :])

            # ---------------- Phase 2: QKV projections ----------------
            # X^T streamed in 4 double-buffered quarters; V/K/Q computed per
            # quarter so compute chases the DMA stream instead of waiting for
            # the full 16 MB.
            with tc.tile_pool(name="xtp", bufs=2) as xtp, \
                 tc.tile_pool(name="psv", bufs=2, space="PSUM") as psv_p, \
                 tc.tile_pool(name="psq", bufs=2, space="PSUM") as psq_p:
                for qtr in range(NSQ):
                    xt_q = xtp.tile([128, NET * SQ], BF16, name="xt_q", tag="xt")
                    for et in range(NET):
                        nc.sync.dma_start(
                            out=xt_q[:, et * SQ : (et + 1) * SQ],
                            in_=XT[et * 128 : (et + 1) * 128, qtr * SQ : (qtr + 1) * SQ],
                        )
                    # V for the 8 s-tiles of this quarter
                    for stl in range(SQ // 128):
                        st = qtr * (SQ // 128) + stl
                        psv = psv_p.tile([128, DL], F32, name="psv", tag="psv")
                        for et in range(NET):
                            nc.tensor.matmul(
                                psv[:, :],
                                lhsT=xt_q[:, et * SQ + stl * 128 : et * SQ + (stl + 1) * 128],
                                rhs=wv_sb[:, et * DL : (et + 1) * DL],
                                start=(et == 0),
                                stop=(et == NET - 1),
                            )
                        nc.vector.scalar_tensor_tensor(
                            out=vt[:, st * DL : (st + 1) * DL],
                            in0=psv[:, :],
                            scalar=1.0,
                            in1=bvb_sb[:, :],
                            op0=mybir.AluOpType.mult,
                            op1=mybir.AluOpType.add,
                        )
                    # K then Q (transposed [d, s]) for the 2 q-blocks here
                    for w_sb, base4, bias_base in ((wk_sb, 2, 2), (wq_sb, 0, 0)):
                        for dt in range(NH):
                            for sbl in range(SQ // QB):
                                sb = qtr * (SQ // QB) + sbl
                                psq = psq_p.tile([128, QB], F32, name="psq", tag="psq")
                                for et in range(NET):
                                    nc.tensor.matmul(
                                        psq[:, :],
                                        lhsT=w_sb[:, et * DL + dt * 128 : et * DL + (dt + 1) * 128],
                                        rhs=xt_q[:, et * SQ + sbl * QB : et * SQ + (sbl + 1) * QB],
                                        start=(et == 0),
                                        stop=(et == NET - 1),
                                    )
                                nc.vector.tensor_scalar_add(
                                    out=qkt[:, (base4 + dt) * S + sb * QB : (base4 + dt) * S + (sb + 1) * QB],
                                    in0=psq[:, :],
                                    scalar1=biasqk[:, bias_base + dt : bias_base + dt + 1],
                                )

            # ---------------- Phases 3+4: attention, then O-projection ----
            with tc.tile_pool(name="mid", bufs=1) as mid, \
                 tc.tile_pool(name="psO", bufs=2, space="PSUM") as psO_p, \
                 tc.tile_pool(name="psD", bufs=2, space="PSUM") as psD_p, \
                 tc.tile_pool(name="psS", bufs=3, space="PSUM") as psS_p, \
                 tc.tile_pool(name="psF", bufs=1, space="PSUM") as psF_p, \
                 tc.tile_pool(name="pp", bufs=10) as pp, \
                 tc.tile_pool(name="rp", bufs=3) as rp, \
                 tc.tile_pool(name="op", bufs=4) as op:
                # normalized attention outputs, transposed: (h*NQB+qb) tile [128d, 512q]
                outt = mid.tile([128, NH * NQB * QB], BF16, name="outt")

                # qb-outer; the two heads' k-groups are interleaved so the
                # scheduler always has a second independent pipeline to fill
                # PE/ACT bubbles. O-projection fused per q-block.
                for qb in range(NQB):
                    kmax = 4 * (qb + 1) if is_causal else NKT
                    groups = list(range(0, kmax, 4))
                    ngroups = len(groups)
                    psO = {}
                    psD = {}
                    for h in range(NH):
                        psO[h] = psO_p.tile([128, QB], F32, name="psO", tag="psO")
                        psD[h] = psD_p.tile([128, QB], F32, name="psD", tag="psD")
                    for gi, kg in enumerate(groups):
                        for h in range(NH):
                            ps_group = []
                            for jj4 in range(4):
                                kt = kg + jj4
                                psS = psS_p.tile([128, QB], F32, name="psS", tag="psS")
                                nc.tensor.matmul(
                                    psS[:, :],
                                    lhsT=qkt[:, (2 + h) * S + kt * 128 : (2 + h) * S + (kt + 1) * 128],
                                    rhs=qkt[:, h * S + qb * QB : h * S + (qb + 1) * QB],
                                    start=True,
                                    stop=True,
                                )
                                p = pp.tile([128, QB], BF16, name="p", tag="p")
                                nc.scalar.activation(
                                    p[:, :], psS[:, :],
                                    mybir.ActivationFunctionType.Exp,
                                    scale=float(SCALE),
                                )
                                if is_causal and kt >= 4 * qb:
                                    # multiplicative 0/1 causal mask on DVE —
                                    # exp(s)*0 == masked, keeps masking off PE
                                    jj = kt - 4 * qb
                                    nc.vector.tensor_mul(
                                        p[:, :], p[:, :],
                                        masks_sb[:, jj * QB : (jj + 1) * QB],
                                    )
                                ps_group.append(p)
                            for jj4 in range(4):
                                kt = kg + jj4
                                nc.tensor.matmul(
                                    psO[h][:, :],
                                    lhsT=vt[:, kt * DL + h * 128 : kt * DL + (h + 1) * 128],
                                    rhs=ps_group[jj4][:, :],
                                    start=(gi == 0 and jj4 == 0),
                                    stop=(gi == ngroups - 1 and jj4 == 3),
                                )
                            # 4 packed denominator matmuls into separate PE
                            # column groups / PSUM rows
                            for jj4 in range(4):
                                nc.tensor.matmul(
                                    psD[h][32 * jj4 : 32 * jj4 + 1, :],
                                    lhsT=ones_col[:, :],
                                    rhs=ps_group[jj4][:, :],
                                    start=(gi == 0),
                                    stop=(gi == ngroups - 1),
                                    tile_position=(0, 32 * jj4),
                                )
                    for h in range(NH):
                        # combine the 4 partial-sum rows, then 1/denom
                        sum4 = rp.tile([1, QB], F32, name="sum4", tag="sum4")
                        nc.vector.tensor_copy(sum4[:, :], psD[h][0:1, :])
                        for jj4 in range(1, 4):
                            nc.vector.scalar_tensor_tensor(
                                out=sum4[:, :],
                                in0=psD[h][32 * jj4 : 32 * jj4 + 1, :],
                                scalar=1.0,
                                in1=sum4[:, :],
                                op0=mybir.AluOpType.mult,
                                op1=mybir.AluOpType.add,
                            )
                        recip = rp.tile([1, QB], F32, name="recip", tag="recip")
                        nc.vector.reciprocal_approx_fast(recip[:, :], sum4[:, :])
                        # broadcast 1/denom across partitions on GpSimd
                        rb = rp.tile([128, QB], F32, name="rb", tag="rb")
                        nc.gpsimd.partition_broadcast(rb[:, :], recip[:, :])
                        o_base = (h * NQB + qb) * QB
                        nc.vector.tensor_mul(
                            outt[:, o_base : o_base + QB], psO[h][:, :], rb[:, :]
                        )

                    # O-projection for this q-block (both heads ready)
                    for j in range(4):
                        st = qb * 4 + j
                        for et in range(4):
                            psF = psF_p.tile([128, 512], F32, name="psF", tag="psF")
                            for h in range(NH):
                                o_base = (h * NQB + qb) * QB + j * 128
                                nc.tensor.matmul(
                                    psF[:, :],
                                    lhsT=outt[:, o_base : o_base + 128],
                                    rhs=wot_sb[:, h * D + et * 512 : h * D + (et + 1) * 512],
                                    start=(h == 0),
                                    stop=(h == NH - 1),
                                )
                            osb = op.tile([128, 512], F32, name="osb", tag="osb")
                            nc.vector.tensor_copy(osb[:, :], psF[:, :])
                            nc.sync.dma_start(
                                out=OUT[st * 128 : (st + 1) * 128, et * 512 : (et + 1) * 512],
                                in_=osb[:, :],
                            )
    nc.finalize()
    return nc


def _bf16(a: np.ndarray) -> np.ndarray:
    return np.ascontiguousarray(a.astype(ml_dtypes.bfloat16))


def make_in_maps(X, Wq, bq, Wk, bk, Wv, bv, Wo, is_causal: bool):
    x2d = np.asarray(X, dtype=np.float32).reshape(S, D)
    xt = _bf16(x2d.T)
    masks = np.zeros((4, 128, QB), dtype=ml_dtypes.bfloat16)
    if is_causal:
        ki = np.arange(128)[:, None]
        qj = np.arange(QB)[None, :]
        for jj in range(4):
            masks[jj] = (128 * jj + ki <= qj).astype(ml_dtypes.bfloat16)

    in_maps = []
    for c in range(NCORES):
        sl = slice(c * DL, (c + 1) * DL)
        in_maps.append(
            {
                "xt": xt,
                "wqt": _bf16(np.asarray(Wq)[sl, :].T),
                "wkt": _bf16(np.asarray(Wk)[sl, :].T),
                "wvt": _bf16(np.asarray(Wv)[sl, :].T),
                "bqkc": np.ascontiguousarray(
                    np.stack(
                        [
                            np.asarray(bq, dtype=np.float32)[sl][:128],
                            np.asarray(bq, dtype=np.float32)[sl][128:],
                            np.asarray(bk, dtype=np.float32)[sl][:128],
                            np.asarray(bk, dtype=np.float32)[sl][128:],
                        ],
                        axis=1,
                    )
                ),
                "bvrow": _bf16(np.asarray(bv)[None, sl]),
                "wot": _bf16(np.asarray(Wo)[:, sl].T),
                "masks": masks,
            }
        )
    return in_maps


_NC_CACHE: dict = {}


def _get_nc(is_causal: bool) -> bass.Bass:
    if is_causal not in _NC_CACHE:
        _NC_CACHE[is_causal] = build_nc(is_causal)
    return _NC_CACHE[is_causal]


def kernel(X, Wq, bq, Wk, bk, Wv, bv, Wo, bo, is_causal, **run_kwargs):
    causal = bool(int(np.asarray(is_causal)))
    nc = _get_nc(causal)
    in_maps = make_in_maps(X, Wq, bq, Wk, bk, Wv, bv, Wo, causal)
    res = run_bass_kernel_spmd(nc, in_maps, core_ids=list(range(NCORES)), **run_kwargs)
    out = np.asarray(bo, dtype=np.float32)[None, :].repeat(S, axis=0)
    for c in range(NCORES):
        out += res.results[c]["out"]
    return out.reshape(1, S, D)
